# revision 5
# baseline (speedup 1.0000x reference)
"""Trainium2 Bass kernel for masked spatial attention softmax.

Computes S = softmax((F_a@Wq.T + bq) @ (F_s@Wk.T + bk).T / sqrt(d) + mask)
over 8 NeuronCores, data-parallel over batch.

Key algebraic restructure: QK = Q_a @ K_s.T = ((F_a@Wq.T + bq) @ Wk) @ F_s.T
+ (Q_a . bk) 1^T.  The bk term is constant along the softmax axis, so it
drops out of the softmax entirely; K_s is never materialized.  This halves
the matmul FLOPs and removes a 4096xd projection per batch.
"""

import math
from contextlib import ExitStack

import numpy as np
import ml_dtypes

import concourse.bass as bass
import concourse.tile as tile
from concourse import bacc, mybir
from concourse.masks import make_identity

# Problem shapes (hardcoded per contract; spec: B=32, T=256, HW=4096, d=256)
B_FULL = 32
N_CORES = 8
BS = B_FULL // N_CORES  # batches per core
T = 256
HW = 4096
D = 256
SCALE = 1.0 / math.sqrt(D)  # 1/16
MASK_NEG = -80.0  # exp(-80 + max_logit) << 1e-30; stays in ACT exp valid range

F32 = mybir.dt.float32
F32R = mybir.dt.float32r
BF16 = mybir.dt.bfloat16


def _build_body(tc, ctx, F_a, F_s, mbig, Wq, Wk, bq, S):
    nc = tc.nc

    singles = ctx.enter_context(tc.tile_pool(name="singles", bufs=1))
    fnat_pool = ctx.enter_context(tc.tile_pool(name="fnat", bufs=2))
    fst_pool = ctx.enter_context(tc.tile_pool(name="fst", bufs=2))
    qpool = ctx.enter_context(tc.tile_pool(name="qpool", bufs=2))
    ppool = ctx.enter_context(tc.tile_pool(name="ppool", bufs=2))
    spool = ctx.enter_context(tc.tile_pool(name="spool", bufs=2))
    stats = ctx.enter_context(tc.tile_pool(name="stats", bufs=4))
    psum_tr = ctx.enter_context(tc.tile_pool(name="psum_tr", bufs=2, space="PSUM"))
    psum_qk = ctx.enter_context(tc.tile_pool(name="psum_qk", bufs=2, space="PSUM"))
    psum_pj = ctx.enter_context(tc.tile_pool(name="psum_pj", bufs=2, space="PSUM"))

    # ---- constants ----
    ident16 = singles.tile([128, 128], BF16, tag="ident16")
    make_identity(nc, ident16[:])
    ones16 = singles.tile([1, 128], BF16, tag="ones16")
    nc.vector.memset(ones16[:], 1.0)

    # mask additive term, all batches: [1, BS*HW] bf16 (0 or MASK_NEG)
    mb_sb = singles.tile([1, BS * HW], BF16, tag="mb")
    nc.sync.dma_start(out=mb_sb[:], in_=mbig.rearrange("b s -> (b s)")[None, :])

    # bq as per-partition scalars: [128, 2] (do-tile major in free dim)
    bq_sb = singles.tile([128, 2], F32, tag="bq")
    nc.sync.dma_start(out=bq_sb[:], in_=bq.rearrange("(a p) -> p a", p=128))

    # Wk natural: lhsT[j, i] for Q~T = sum_j Wk[j,i] Q.T[j,t]  (bf16 cast DMA)
    wk_sb = singles.tile([128, 2, D], BF16, tag="wk")
    nc.gpsimd.dma_start(out=wk_sb[:], in_=Wk.rearrange("(jh jl) i -> jl jh i", jl=128))

    # Wq loaded natural (bf16), then PE-transposed to WqT[di, do]
    wq_nat = singles.tile([128, 2, D], BF16, tag="wqn")
    nc.gpsimd.dma_start(
        out=wq_nat[:], in_=Wq.rearrange("(oh ol) i -> ol oh i", ol=128)
    )
    wqt = singles.tile([128, 2, D], BF16, tag="wqt")
    for k in range(2):  # di tile
        pj = psum_pj.tile([128, D], BF16, tag="pj")
        for m in range(2):  # do tile
            nc.tensor.matmul(
                pj[:, m * 128:(m + 1) * 128],
                wq_nat[:, m, k * 128:(k + 1) * 128],
                ident16[:],
                is_transpose=True,
                start=(m == 0),
                stop=(m == 1),
            )
        nc.vector.tensor_copy(out=wqt[:, k, :], in_=pj[:])

    for b in range(BS):
        # ---- F_s natural load (fp32 -> bf16 cast in SWDGE DMA) ----
        # layout [sl, sh, c] with s = sh*128 + sl
        fnat = fnat_pool.tile([128, 32, D], BF16, tag="fnat")
        nc.gpsimd.dma_start(
            out=fnat[:], in_=F_s[b].rearrange("(sh sl) c -> sl sh c", sl=128)
        )

        # ---- F_sT via PE transposes: [c_l, c_tile, s] ----
        fst = fst_pool.tile([128, 2, HW], BF16, tag="fst")
        for ci in range(2):
            for o in range(4):  # octet of sh values -> fills one PSUM bank
                pt = psum_tr.tile([128, 8, 128], BF16, tag="pt")
                for k in range(8):
                    sh = o * 8 + k
                    nc.tensor.matmul(
                        pt[:, k, :],
                        fnat[:, sh, ci * 128:(ci + 1) * 128],
                        ident16[:],
                        is_transpose=True,
                        start=(k == 0),
                        stop=(k == 7),
                    )
                nc.vector.tensor_copy(
                    out=fst[:, ci, o * 1024:(o + 1) * 1024],
                    in_=pt[:].rearrange("p a b -> p (a b)"),
                )

        # ---- F_a load + transpose (bf16) ----
        fa = qpool.tile([128, 2, D], BF16, tag="fa")  # [tl, th, d]
        nc.gpsimd.dma_start(
            out=fa[:], in_=F_a[b].rearrange("(th tl) d -> tl th d", tl=128)
        )
        fat = qpool.tile([128, 2, T], BF16, tag="fat")  # [d_l, d_tile, t]
        for k in range(2):  # d tile
            pj = psum_pj.tile([128, T], BF16, tag="pj")
            for m in range(2):  # t tile
                nc.tensor.matmul(
                    pj[:, m * 128:(m + 1) * 128],
                    fa[:, m, k * 128:(k + 1) * 128],
                    ident16[:],
                    is_transpose=True,
                    start=(m == 0),
                    stop=(m == 1),
                )
            nc.vector.tensor_copy(out=fat[:, k, :], in_=pj[:])

        # ---- Q.T = Wq @ F_a.T + bq ----
        qt = qpool.tile([128, 2, T], BF16, tag="qt")  # [do_l, do_tile, t]
        for m in range(2):  # do tile
            pj = psum_pj.tile([128, T], F32, tag="pj")
            for k in range(2):  # di tile
                nc.tensor.matmul(
                    pj[:],
                    wqt[:, k, m * 128:(m + 1) * 128],
                    fat[:, k, :],
                    start=(k == 0),
                    stop=(k == 1),
                )
            nc.vector.tensor_scalar_add(
                out=qt[:, m, :], in0=pj[:], scalar1=bq_sb[:, m:m + 1]
            )

        # ---- Q~T = Wk @ Q.T, scaled by 1/sqrt(d), cast to bf16 ----
        qct = qpool.tile([128, 2, T], BF16, tag="qct")  # [i_l, i_tile, t]
        for m in range(2):  # i tile
            pj = psum_pj.tile([128, T], F32, tag="pj")
            for k in range(2):  # j tile
                nc.tensor.matmul(
                    pj[:],
                    wk_sb[:, k, m * 128:(m + 1) * 128],
                    qt[:, k, :],
                    start=(k == 0),
                    stop=(k == 1),
                )
            nc.vector.tensor_scalar_mul(out=qct[:, m, :], in0=pj[:], scalar1=SCALE)

        # ---- QK + mask (in PE), exp (ACT, PSUM->SBUF), normalize (DVE) ----
        for tt in range(2):  # row tile of T
            p_tile = ppool.tile([128, HW], F32, tag="p")
            st = stats.tile([128, 4], F32, tag="st")
            for ck in range(4):  # 1024-wide chunks (2 PSUM banks)
                pq = psum_qk.tile([128, 1024], F32, tag="pq")
                for h in range(2):  # 512-wide half = one PSUM bank
                    s0 = ck * 1024 + h * 512
                    for ci in range(2):
                        nc.tensor.matmul(
                            pq[:, h * 512:(h + 1) * 512],
                            qct[:, ci, tt * 128:(tt + 1) * 128],
                            fst[:, ci, s0:s0 + 512],
                            start=(ci == 0),
                            stop=False,
                        )
                    nc.tensor.matmul(
                        pq[:, h * 512:(h + 1) * 512],
                        ones16[:],
                        mb_sb[:, b * HW + s0: b * HW + s0 + 512],
                        start=False,
                        stop=True,
                    )
                nc.scalar.activation(
                    out=p_tile[:, ck * 1024:(ck + 1) * 1024],
                    in_=pq[:],
                    func=mybir.ActivationFunctionType.Exp,
                    accum_out=st[:, ck:ck + 1],
                )
            rowsum = stats.tile([128, 1], F32, tag="rowsum")
            nc.vector.reduce_sum(out=rowsum[:], in_=st[:], axis=mybir.AxisListType.X)
            recip = stats.tile([128, 1], F32, tag="recip")
            nc.vector.reciprocal(out=recip[:], in_=rowsum[:])
            s_tile = spool.tile([128, HW], F32, tag="s")
            nc.vector.tensor_scalar_mul(
                out=s_tile[:], in0=p_tile[:], scalar1=recip[:, 0:1]
            )
            nc.sync.dma_start(out=S[b, tt * 128:(tt + 1) * 128, :], in_=s_tile[:])


def build_nc():
    nc = bacc.Bacc(
        "TRN2",
        target_bir_lowering=False,
        debug=False,
        num_devices=N_CORES,
    )
    F_a = nc.dram_tensor("F_a", [BS, T, D], F32, kind="ExternalInput")
    F_s = nc.dram_tensor("F_s", [BS, HW, D], F32, kind="ExternalInput")
    mbig = nc.dram_tensor("mbig", [BS, HW], BF16, kind="ExternalInput")
    Wq = nc.dram_tensor("Wq", [D, D], F32, kind="ExternalInput")
    Wk = nc.dram_tensor("Wk", [D, D], F32, kind="ExternalInput")
    bq = nc.dram_tensor("bq", [D], F32, kind="ExternalInput")
    S = nc.dram_tensor("S", [BS, T, HW], F32, kind="ExternalOutput")

    with tile.TileContext(nc) as tc, ExitStack() as ctx:
        _build_body(
            tc, ctx, F_a.ap(), F_s.ap(), mbig.ap(), Wq.ap(), Wk.ap(), bq.ap(), S.ap()
        )
    nc.compile()
    return nc


def make_in_maps(F_a, F_s, M_s, Wq, bq, Wk):
    F_a = np.asarray(F_a, dtype=np.float32)
    F_s = np.asarray(F_s, dtype=np.float32)
    M_s = np.asarray(M_s)
    Wq = np.ascontiguousarray(np.asarray(Wq, dtype=np.float32))
    Wk = np.ascontiguousarray(np.asarray(Wk, dtype=np.float32))
    bq = np.ascontiguousarray(np.asarray(bq, dtype=np.float32))

    m = M_s.reshape(M_s.shape[0], -1) == 1  # [B, HW]
    mbig = np.where(m, np.float32(0.0), np.float32(MASK_NEG)).astype(
        ml_dtypes.bfloat16
    )

    in_maps = []
    for i in range(N_CORES):
        sl = slice(i * BS, (i + 1) * BS)
        in_maps.append(
            dict(
                F_a=np.ascontiguousarray(F_a[sl]),
                F_s=np.ascontiguousarray(F_s[sl]),
                mbig=np.ascontiguousarray(mbig[sl]),
                Wq=Wq,
                Wk=Wk,
                bq=bq,
            )
        )
    return in_maps


_NC_CACHE = None


def _get_nc():
    global _NC_CACHE
    if _NC_CACHE is None:
        _NC_CACHE = build_nc()
    return _NC_CACHE


def run(in_maps, **kwargs):
    from concourse import bass_utils

    nc = _get_nc()
    res = bass_utils.run_bass_kernel_spmd(
        nc, in_maps, core_ids=list(range(N_CORES)), **kwargs
    )
    return res


def kernel(F_a, F_s, M_s, Wq, bq, Wk, bk):
    in_maps = make_in_maps(F_a, F_s, M_s, Wq, bq, Wk)
    res = run(in_maps)
    return np.concatenate([r["S"] for r in res.results], axis=0)


# revision 8
# speedup vs baseline: 1.1207x; 1.1207x over previous
"""Trainium2 Bass kernel for masked spatial attention softmax.

Computes S = softmax((F_a@Wq.T + bq) @ (F_s@Wk.T + bk).T / sqrt(d) + mask)
over 8 NeuronCores, data-parallel over batch.

Key algebraic restructure: QK = Q_a @ K_s.T = ((F_a@Wq.T + bq) @ Wk) @ F_s.T
+ (Q_a . bk) 1^T.  The bk term is constant along the softmax axis, so it
drops out of the softmax entirely; K_s is never materialized.  This halves
the matmul FLOPs and removes a 4096xd projection per batch.

Software pipeline: batch b's QK/exp phase interleaves batch b+1's F_s
transposes on the PE stream (keeps PE dense, HAM warm); loads prefetch
two batches ahead.
"""

import math
from contextlib import ExitStack

import numpy as np
import ml_dtypes

import concourse.bass as bass
import concourse.tile as tile
from concourse import bacc, mybir
from concourse.masks import make_identity

# Problem shapes (hardcoded per contract; spec: B=32, T=256, HW=4096, d=256)
B_FULL = 32
N_CORES = 8
BS = B_FULL // N_CORES  # batches per core
T = 256
HW = 4096
D = 256
SCALE = 1.0 / math.sqrt(D)  # 1/16
MASK_NEG = -80.0  # exp(-80 + max_logit) << 1e-30; stays in ACT exp valid range

F32 = mybir.dt.float32
BF16 = mybir.dt.bfloat16


def _build_body(tc, ctx, F_a, F_s, mbig, Wq, Wk, bq, S):
    nc = tc.nc

    singles = ctx.enter_context(tc.tile_pool(name="singles", bufs=1))
    fnat_pool = ctx.enter_context(tc.tile_pool(name="fnat", bufs=2))
    fst_pool = ctx.enter_context(tc.tile_pool(name="fst", bufs=2))
    qpool = ctx.enter_context(tc.tile_pool(name="qpool", bufs=2))
    ppool = ctx.enter_context(tc.tile_pool(name="ppool", bufs=2))
    stats = ctx.enter_context(tc.tile_pool(name="stats", bufs=4))
    psum_tr = ctx.enter_context(tc.tile_pool(name="psum_tr", bufs=2, space="PSUM"))
    psum_qk = ctx.enter_context(tc.tile_pool(name="psum_qk", bufs=2, space="PSUM"))
    psum_pj = ctx.enter_context(tc.tile_pool(name="psum_pj", bufs=2, space="PSUM"))

    # ---- constants ----
    ident16 = singles.tile([128, 128], BF16, tag="ident16", name="ident16")
    make_identity(nc, ident16[:])
    ones16 = singles.tile([1, 128], BF16, tag="ones16", name="ones16")
    nc.vector.memset(ones16[:], 1.0)

    # mask additive term, all batches: [1, BS*HW] bf16 (0 or MASK_NEG)
    mb_sb = singles.tile([1, BS * HW], BF16, tag="mb", name="mb")
    nc.sync.dma_start(out=mb_sb[:], in_=mbig.rearrange("b s -> (b s)")[None, :])

    # bq as per-partition scalars: [128, 2] (do-tile major in free dim)
    bq_sb = singles.tile([128, 2], F32, tag="bq", name="bq")
    nc.sync.dma_start(out=bq_sb[:], in_=bq.rearrange("(a p) -> p a", p=128))

    # Wk natural: lhsT[j, i] for Q~T = sum_j Wk[j,i] Q.T[j,t]  (bf16 cast DMA)
    wk_sb = singles.tile([128, 2, D], BF16, tag="wk", name="wk")
    nc.gpsimd.dma_start(out=wk_sb[:], in_=Wk.rearrange("(jh jl) i -> jl jh i", jl=128))

    # Wq loaded natural (bf16), then PE-transposed to WqT[di, do]
    wq_nat = singles.tile([128, 2, D], BF16, tag="wqn", name="wqn")
    nc.gpsimd.dma_start(
        out=wq_nat[:], in_=Wq.rearrange("(oh ol) i -> ol oh i", ol=128)
    )
    wqt = singles.tile([128, 2, D], BF16, tag="wqt", name="wqt")
    for k in range(2):  # di tile
        pj = psum_pj.tile([128, D], BF16, tag="pj", name="pj")
        for m in range(2):  # do tile
            nc.tensor.matmul(
                pj[:, m * 128:(m + 1) * 128],
                wq_nat[:, m, k * 128:(k + 1) * 128],
                ident16[:],
                is_transpose=True,
                start=(m == 0),
                stop=(m == 1),
            )
        nc.vector.tensor_copy(out=wqt[:, k, :], in_=pj[:])

    fa_t, fnat_t, fst_t, qct_t = {}, {}, {}, {}

    def load_batch(b):
        """Prefetch F_a[b] (small, first) and F_s[b] in halves (SWDGE casts)."""
        fa = qpool.tile([128, 2, D], BF16, tag="fa", name="fa")  # [tl, th, d]
        nc.gpsimd.dma_start(
            out=fa[:], in_=F_a[b].rearrange("(th tl) d -> tl th d", tl=128)
        )
        fa_t[b] = fa
        fnat = fnat_pool.tile([128, 32, D], BF16, tag="fnat", name="fnat")  # [sl, sh, c]
        src = F_s[b].rearrange("(sh sl) c -> sl sh c", sl=128)
        for h in range(2):
            nc.gpsimd.dma_start(
                out=fnat[:, h * 16:(h + 1) * 16, :], in_=src[:, h * 16:(h + 1) * 16, :]
            )
        fnat_t[b] = fnat

    def qchain(b):
        """F_a.T -> Q.T -> Q~T (bf16, tiny)."""
        fa = fa_t.pop(b)
        fat = qpool.tile([128, 2, T], BF16, tag="fat", name="fat")  # [d_l, d_tile, t]
        for k in range(2):  # d tile
            pj = psum_pj.tile([128, T], BF16, tag="pj", name="pj")
            for m in range(2):  # t tile
                nc.tensor.matmul(
                    pj[:, m * 128:(m + 1) * 128],
                    fa[:, m, k * 128:(k + 1) * 128],
                    ident16[:],
                    is_transpose=True,
                    start=(m == 0),
                    stop=(m == 1),
                )
            nc.vector.tensor_copy(out=fat[:, k, :], in_=pj[:])

        qt = qpool.tile([128, 2, T], BF16, tag="qt", name="qt")  # [do_l, do_tile, t]
        for m in range(2):  # do tile
            pj = psum_pj.tile([128, T], F32, tag="pj", name="pj")
            for k in range(2):  # di tile
                nc.tensor.matmul(
                    pj[:],
                    wqt[:, k, m * 128:(m + 1) * 128],
                    fat[:, k, :],
                    start=(k == 0),
                    stop=(k == 1),
                )
            nc.vector.tensor_scalar_add(
                out=qt[:, m, :], in0=pj[:], scalar1=bq_sb[:, m:m + 1]
            )

        qct = qpool.tile([128, 2, T], BF16, tag="qct", name="qct")  # [i_l, i_tile, t]
        for m in range(2):  # i tile
            pj = psum_pj.tile([128, T], F32, tag="pj", name="pj")
            for k in range(2):  # j tile
                nc.tensor.matmul(
                    pj[:],
                    wk_sb[:, k, m * 128:(m + 1) * 128],
                    qt[:, k, :],
                    start=(k == 0),
                    stop=(k == 1),
                )
            nc.vector.tensor_scalar_mul(out=qct[:, m, :], in0=pj[:], scalar1=SCALE)
        qct_t[b] = qct

    def transpose_octet(b, ci, o):
        """8 PE transposes of [128,128] bf16 into one PSUM bank, one eviction."""
        fnat = fnat_t[b]
        fst = fst_t[b]
        pt = psum_tr.tile([128, 8, 128], BF16, tag="pt", name="pt")
        for k in range(8):
            sh = o * 8 + k
            nc.tensor.matmul(
                pt[:, k, :],
                fnat[:, sh, ci * 128:(ci + 1) * 128],
                ident16[:],
                is_transpose=True,
                start=(k == 0),
                stop=(k == 7),
            )
        nc.vector.tensor_copy(
            out=fst[:, ci, o * 1024:(o + 1) * 1024],
            in_=pt[:].rearrange("p a b -> p (a b)"),
        )

    def qk_chunk(b, tt, ck, p_tile, st):
        """QK + mask for one [128, 1024] chunk (2 PSUM banks), then exp."""
        fst = fst_t[b]
        qct = qct_t[b]
        pq = psum_qk.tile([128, 1024], F32, tag="pq", name="pq")
        for h in range(2):  # 512-wide half = one PSUM bank
            s0 = ck * 1024 + h * 512
            for ci in range(2):
                nc.tensor.matmul(
                    pq[:, h * 512:(h + 1) * 512],
                    qct[:, ci, tt * 128:(tt + 1) * 128],
                    fst[:, ci, s0:s0 + 512],
                    start=(ci == 0),
                    stop=False,
                )
            nc.tensor.matmul(
                pq[:, h * 512:(h + 1) * 512],
                ones16[:],
                mb_sb[:, b * HW + s0: b * HW + s0 + 512],
                start=False,
                stop=True,
            )
        nc.scalar.activation(
            out=p_tile[:, ck * 1024:(ck + 1) * 1024],
            in_=pq[:],
            func=mybir.ActivationFunctionType.Exp,
            accum_out=st[:, ck:ck + 1],
        )

    def finish_rowtile(b, tt, p_tile, st):
        rowsum = stats.tile([128, 1], F32, tag="rowsum", name="rowsum")
        nc.vector.reduce_sum(out=rowsum[:], in_=st[:], axis=mybir.AxisListType.X)
        recip = stats.tile([128, 1], F32, tag="recip", name="recip")
        nc.vector.reciprocal(out=recip[:], in_=rowsum[:])
        # in-place normalize, then store
        nc.vector.tensor_scalar_mul(
            out=p_tile[:], in0=p_tile[:], scalar1=recip[:, 0:1]
        )
        nc.sync.dma_start(out=S[b, tt * 128:(tt + 1) * 128, :], in_=p_tile[:])

    OCTETS = [(ci, o) for ci in range(2) for o in range(4)]

    # ---- software pipeline ----
    load_batch(0)
    qchain(0)
    fst_t[0] = fst_pool.tile([128, 2, HW], BF16, tag="fst", name="fst")
    for ci, o in OCTETS:
        transpose_octet(0, ci, o)
    load_batch(1)
    qchain(1)

    for b in range(BS):
        if b + 2 < BS:
            load_batch(b + 2)
        if b + 1 < BS:
            fst_t[b + 1] = fst_pool.tile([128, 2, HW], BF16, tag="fst", name="fst")
        oi = 0
        for tt in range(2):
            p_tile = ppool.tile([128, HW], F32, tag="p", name="p")
            st = stats.tile([128, 4], F32, tag="st", name="st")
            for ck in range(4):
                qk_chunk(b, tt, ck, p_tile, st)
                if b + 1 < BS:
                    transpose_octet(b + 1, *OCTETS[oi])
                    oi += 1
            finish_rowtile(b, tt, p_tile, st)
        fnat_t.pop(b, None)
        fst_t.pop(b, None)
        qct_t.pop(b, None)
        if b + 2 < BS:
            qchain(b + 2)


def build_nc():
    nc = bacc.Bacc(
        "TRN2",
        target_bir_lowering=False,
        debug=False,
        num_devices=N_CORES,
    )
    F_a = nc.dram_tensor("F_a", [BS, T, D], F32, kind="ExternalInput")
    F_s = nc.dram_tensor("F_s", [BS, HW, D], F32, kind="ExternalInput")
    mbig = nc.dram_tensor("mbig", [BS, HW], BF16, kind="ExternalInput")
    Wq = nc.dram_tensor("Wq", [D, D], F32, kind="ExternalInput")
    Wk = nc.dram_tensor("Wk", [D, D], F32, kind="ExternalInput")
    bq = nc.dram_tensor("bq", [D], F32, kind="ExternalInput")
    S = nc.dram_tensor("S", [BS, T, HW], F32, kind="ExternalOutput")

    with tile.TileContext(nc) as tc, ExitStack() as ctx:
        _build_body(
            tc, ctx, F_a.ap(), F_s.ap(), mbig.ap(), Wq.ap(), Wk.ap(), bq.ap(), S.ap()
        )
    nc.compile()
    return nc


def make_in_maps(F_a, F_s, M_s, Wq, bq, Wk):
    F_a = np.asarray(F_a, dtype=np.float32)
    F_s = np.asarray(F_s, dtype=np.float32)
    M_s = np.asarray(M_s)
    Wq = np.ascontiguousarray(np.asarray(Wq, dtype=np.float32))
    Wk = np.ascontiguousarray(np.asarray(Wk, dtype=np.float32))
    bq = np.ascontiguousarray(np.asarray(bq, dtype=np.float32))

    m = M_s.reshape(M_s.shape[0], -1) == 1  # [B, HW]
    mbig = np.where(m, np.float32(0.0), np.float32(MASK_NEG)).astype(
        ml_dtypes.bfloat16
    )

    in_maps = []
    for i in range(N_CORES):
        sl = slice(i * BS, (i + 1) * BS)
        in_maps.append(
            dict(
                F_a=np.ascontiguousarray(F_a[sl]),
                F_s=np.ascontiguousarray(F_s[sl]),
                mbig=np.ascontiguousarray(mbig[sl]),
                Wq=Wq,
                Wk=Wk,
                bq=bq,
            )
        )
    return in_maps


_NC_CACHE = None


def _get_nc():
    global _NC_CACHE
    if _NC_CACHE is None:
        _NC_CACHE = build_nc()
    return _NC_CACHE


def run(in_maps, **kwargs):
    from concourse import bass_utils

    nc = _get_nc()
    res = bass_utils.run_bass_kernel_spmd(
        nc, in_maps, core_ids=list(range(N_CORES)), **kwargs
    )
    return res


def kernel(F_a, F_s, M_s, Wq, bq, Wk, bk):
    in_maps = make_in_maps(F_a, F_s, M_s, Wq, bq, Wk)
    res = run(in_maps)
    return np.concatenate([r["S"] for r in res.results], axis=0)
